# revision 2
# baseline (speedup 1.0000x reference)
"""Two-layer GAT (8-head + 1-head) Trainium2 Bass kernel, 8-way node-sharded.

Strategy (per core c, owning row block I_c of R = N/8 nodes), layer 1:
  * Softmax over neighbors j is invariant to per-row (per-i) scaling, so
    P[j, i] ~ adjT[j, i] * max(exp((1-a)fs_i + fd_j), exp(a * fd_j))
    using exp(leakyrelu(z)) = max(exp(z), exp(a z)) and dropping the
    exp(a fs_i) row factor.  Two routes build the masked field, balanced
    across ScalarE and VectorE:
      ACT:  t1 = Exp(0.8 fsb + fd_j) (ScalarE, fused bias), then one
            fused DVE op  pp = (t1 max vd_j) * adjT  (scalar_tensor_tensor).
      DVE:  u = abc_i * efd_j (tensor_scalar), then the same fused stt.
  * Scores live in TRANSPOSED layout [j (partition), i (free)] so both
    the aggregation out^T[o, i] = sum_j h[j, o] P[j, i] and the softmax
    denominator Z contract over the partition dim.  Z rides an all-ones
    stationary matmul, landing broadcast across all 128 partitions.
  * h = x @ W1 for all nodes is computed locally (replicated); fd rides
    the same stationary x-chunk as tiny N=8 matmuls against the folded
    W1 @ a_dst vector (no DVE reduction needed).
  * Layer-2 inputs h2 = h1 @ W2 (+ f_src2/f_dst2 via folded W2 columns)
    are tiny; each core computes its own block packed as [128, 72] and an
    AllGather distributes them; layer 2 repeats the same scheme with a
    single head and an 18-column stationary whose col 16 is memset to 1
    so the aggregation's row 16 yields Z2.
  * All host->device tensors are packed partition-major so every DMA is
    a few large contiguous-row transfers.
"""

import sys

sys.path.insert(0, "/opt/trn_rl_repo")

import numpy as np
import ml_dtypes

N = 4096
F_IN = 512
H1 = 8
D1 = 128
F1 = 1024          # H1 * D1
D2 = 16
NCORES = 8
R = N // NCORES    # rows (nodes) per core
NCH = N // 128     # j-chunks of 128
NFC = F_IN // 128  # f chunks
ALPHA = 0.2

_BUILD_CACHE = {}


def _act_route(c, k):
    # route mix for layer-1 score field: True -> ScalarE exp path
    return (c + k) % 2 == 0


def _act_route2(c):
    # route mix for layer-2 score field
    return c % 2 == 0


def _build_nc():
    import concourse.bacc as bacc
    import concourse.tile as tile
    import concourse.mybir as mybir

    FP32 = mybir.dt.float32
    BF16 = mybir.dt.bfloat16
    AF = mybir.ActivationFunctionType
    OP = mybir.AluOpType
    AX = mybir.AxisListType

    nc = bacc.Bacc(num_devices=NCORES)

    # ---- I/O (all packed [128, *] with contiguous rows) ------------------
    xP_d = nc.dram_tensor("xP", [128, NFC * N], BF16, kind="ExternalInput")
    xoP_d = nc.dram_tensor("xoP", [128, NFC * R], BF16, kind="ExternalInput")
    W1P_d = nc.dram_tensor("W1P", [128, NFC * F1], BF16, kind="ExternalInput")
    wf_d = nc.dram_tensor("wfold", [128, NFC * 16], BF16, kind="ExternalInput")
    adjP_d = nc.dram_tensor("adjP", [128, NCH * R], BF16, kind="ExternalInput")
    onesb_d = nc.dram_tensor("onesb", [128, 128], BF16, kind="ExternalInput")
    idb_d = nc.dram_tensor("idb", [128, 128], BF16, kind="ExternalInput")
    idf_d = nc.dram_tensor("idf16", [16, 16], FP32, kind="ExternalInput")
    W2aP_d = nc.dram_tensor("W2aP", [128, H1 * 18], BF16, kind="ExternalInput")
    out_d = nc.dram_tensor("out", [128, (R // 128) * D2], FP32,
                           kind="ExternalOutput")

    with tile.TileContext(nc) as tc:
        with (
            tc.tile_pool(name="const", bufs=1) as cpool,
            tc.tile_pool(name="dram", bufs=1, space="DRAM") as dpool,
        ):
            # ---- resident SBUF tensors ----------------------------------
            W1_sb = cpool.tile([128, NFC * F1], BF16, tag="W1")
            wf_sb = cpool.tile([128, NFC * 16], BF16, tag="wf")
            xo_sb = cpool.tile([128, NFC * R], BF16, tag="xo")
            onesb_sb = cpool.tile([128, 128], BF16, tag="onesb")
            idb_sb = cpool.tile([128, 128], BF16, tag="idb")
            idf_sb = cpool.tile([16, 16], FP32, tag="idf")
            W2a_sb = cpool.tile([128, H1 * 18], BF16, tag="W2a")
            adjT_sb = cpool.tile([128, NCH * R], BF16, tag="adjT")

            h_sb = cpool.tile([128, NCH * F1], BF16, tag="h")
            fsb = cpool.tile([128, H1 * R], BF16, tag="fsb")
            abc = cpool.tile([128, H1 * R], BF16, tag="abc")
            fsx = cpool.tile([128, 2 * R], BF16, tag="fsx")
            fd_sb = cpool.tile([128, NCH * 8], FP32, tag="fd")
            efd_sb = cpool.tile([128, NCH * 8], FP32, tag="efd")
            vd_sb = cpool.tile([128, NCH * 8], FP32, tag="vd")
            h1T = cpool.tile([128, H1 * R], BF16, tag="h1T")
            h2P_sb = cpool.tile([128, (R // 128) * 18], BF16, tag="h2P")
            h2all_sb = cpool.tile([128, NCH * 18], BF16, tag="h2all")
            fs2row = cpool.tile([1, R], FP32, tag="fs2row")
            fsb2 = cpool.tile([128, R], BF16, tag="fsb2")
            abc2 = cpool.tile([128, R], BF16, tag="abc2")
            fd2_sb = cpool.tile([128, NCH], FP32, tag="fd2")
            vd2_sb = cpool.tile([128, NCH], FP32, tag="vd2")
            efd2_sb = cpool.tile([128, NCH], FP32, tag="efd2")
            outbuf = cpool.tile([128, (R // 128) * D2], FP32, tag="outbuf")

            h2loc = dpool.tile([128, (R // 128) * 18], BF16, tag="h2loc")
            h2all_d = dpool.tile([N // 128 * 128, 18], BF16, tag="h2all",
                                 addr_space="Shared")
            # gathered as [8 cores * 128, (R//128)*18] -> reinterpret below
            h2allP_d = dpool.tile([NCORES * 128, (R // 128) * 18], BF16,
                                  tag="h2allP", addr_space="Shared")

            # ---- input DMAs, in order of first use ----------------------
            nc.sync.dma_start(onesb_sb[:], onesb_d[:])
            nc.sync.dma_start(xo_sb[:], xoP_d[:])
            nc.sync.dma_start(wf_sb[:], wf_d[:])
            nc.sync.dma_start(W1_sb[:], W1P_d[:])

            # =============================================================
            # Phase A: own-block f_src -> broadcast rows fsb / abc
            # =============================================================
            with (
                tc.tile_pool(name="pfs", bufs=1, space="PSUM") as pfsp,
                tc.tile_pool(name="pab", bufs=2, space="PSUM") as pabp,
                tc.tile_pool(name="a8p", bufs=1) as a8p,
            ):
                fsT8 = pfsp.tile([8, R], FP32, tag="fs8")
                for fc in range(NFC):
                    nc.tensor.matmul(
                        fsT8[:],
                        wf_sb[:, fc * 16:fc * 16 + 8],
                        xo_sb[:, fc * R:(fc + 1) * R],
                        start=fc == 0, stop=fc == NFC - 1,
                    )
                fs8 = a8p.tile([8, R], BF16, tag="fs8s")
                nc.scalar.activation(fs8[:], fsT8[:], AF.Copy)
                for g in range(H1):
                    q, hf = g % 4, g // 4
                    nc.sync.dma_start(
                        fsx[32 * q:32 * q + 1, hf * R:(hf + 1) * R],
                        fs8[g:g + 1, :],
                    )
                for g in range(H1):
                    q, hf = g % 4, g // 4
                    pb = pabp.tile([128, R], FP32, tag="pab")
                    nc.tensor.matmul(
                        pb[:],
                        onesb_sb[32 * q:32 * q + 1, :],
                        fsx[32 * q:32 * q + 1, hf * R:(hf + 1) * R],
                        start=True, stop=True, tile_position=(32 * q, 0),
                    )
                    nc.scalar.activation(
                        fsb[:, g * R:(g + 1) * R], pb[:], AF.Copy
                    )
                    nc.scalar.activation(
                        abc[:, g * R:(g + 1) * R], pb[:], AF.Exp,
                        scale=1.0 - ALPHA,
                    )

            # =============================================================
            # Phase B: h = x @ W1 (all nodes) + fd rider matmuls
            # =============================================================
            with tc.tile_pool(name="xres", bufs=1) as xrp:
                x_sb = xrp.tile([128, NFC * N], BF16, tag="x")
                for q in range(4):
                    nc.sync.dma_start(
                        x_sb[:, q * 4096:(q + 1) * 4096],
                        xP_d[:, q * 4096:(q + 1) * 4096],
                    )
                for q in range(4):
                    nc.sync.dma_start(
                        adjT_sb[:, q * 8 * R:(q + 1) * 8 * R],
                        adjP_d[:, q * 8 * R:(q + 1) * 8 * R],
                    )
                nc.sync.dma_start(idb_sb[:], idb_d[:])
                nc.sync.dma_start(idf_sb[:], idf_d[:])
                nc.sync.dma_start(W2a_sb[:], W2aP_d[:])

                with (
                    tc.tile_pool(name="ph", bufs=3, space="PSUM") as php,
                    tc.tile_pool(name="pfd", bufs=1, space="PSUM") as pfdp,
                ):
                    pfd = pfdp.tile([128, NCH * 8], FP32, tag="pfd")
                    for jt in range(NCH):
                        q, j8 = jt // 8, jt % 8
                        ph = php.tile([128, F1], FP32, tag="ph")
                        for fc in range(NFC):
                            lhs = x_sb[:, q * 4096 + fc * 1024 + j8 * 128:
                                       q * 4096 + fc * 1024 + (j8 + 1) * 128]
                            st, sp = fc == 0, fc == NFC - 1
                            nc.tensor.matmul(
                                ph[:, 0:512], lhs,
                                W1_sb[:, fc * F1:fc * F1 + 512],
                                start=st, stop=sp,
                            )
                            nc.tensor.matmul(
                                ph[:, 512:F1], lhs,
                                W1_sb[:, fc * F1 + 512:(fc + 1) * F1],
                                start=st, stop=sp,
                            )
                            nc.tensor.matmul(
                                pfd[:, jt * 8:(jt + 1) * 8], lhs,
                                wf_sb[:, fc * 16 + 8:fc * 16 + 16],
                                start=st, stop=sp,
                            )
                        nc.scalar.activation(
                            h_sb[:, jt * F1:jt * F1 + 512], ph[:, 0:512], AF.Copy
                        )
                        nc.scalar.activation(
                            h_sb[:, jt * F1 + 512:(jt + 1) * F1], ph[:, 512:F1],
                            AF.Copy,
                        )
                        if jt % 8 == 7:
                            # quarter's fd ready: extract fd / e^fd / e^(a fd)
                            qs = (jt - 7) * 8
                            qe = (jt + 1) * 8
                            nc.scalar.activation(
                                fd_sb[:, qs:qe], pfd[:, qs:qe], AF.Copy
                            )
                            nc.scalar.activation(
                                efd_sb[:, qs:qe], pfd[:, qs:qe], AF.Exp
                            )
                            nc.scalar.activation(
                                vd_sb[:, qs:qe], pfd[:, qs:qe], AF.Exp,
                                scale=ALPHA,
                            )

            # =============================================================
            # Phase C: layer-1 attention, 2 heads (one pair) per pass
            # =============================================================
            with (
                tc.tile_pool(name="acc", bufs=2, space="PSUM") as acc,
                tc.tile_pool(name="sc", bufs=5) as spool,
                tc.tile_pool(name="nrm", bufs=2) as npool,
            ):
                for p in range(4):
                    g0, g1 = 2 * p, 2 * p + 1
                    po0 = acc.tile([128, R], FP32, tag="po0", name="po0")
                    po1 = acc.tile([128, R], FP32, tag="po1", name="po1")
                    pz = acc.tile([128, 2 * R], FP32, tag="pz", name="pz")
                    for c in range(NCH):
                        pp = spool.tile([128, 2 * R], BF16, tag="pp")
                        for k, g in ((0, g0), (1, g1)):
                            sidx = c * 8 + g
                            if _act_route(c, k):
                                t1 = spool.tile([128, R], BF16, tag="t1")
                                nc.scalar.activation(
                                    t1[:],
                                    fsb[:, g * R:(g + 1) * R],
                                    AF.Exp,
                                    bias=fd_sb[:, sidx:sidx + 1],
                                    scale=1.0 - ALPHA,
                                )
                                src = t1
                            else:
                                u = spool.tile([128, R], BF16, tag="t1")
                                nc.vector.tensor_scalar_mul(
                                    u[:],
                                    abc[:, g * R:(g + 1) * R],
                                    efd_sb[:, sidx:sidx + 1],
                                )
                                src = u
                            nc.vector.scalar_tensor_tensor(
                                pp[:, k * R:(k + 1) * R],
                                src[:],
                                vd_sb[:, sidx:sidx + 1],
                                adjT_sb[:, c * R:(c + 1) * R],
                                op0=OP.max, op1=OP.mult,
                            )
                        nc.tensor.matmul(
                            po0[:],
                            h_sb[:, c * F1 + g0 * D1:c * F1 + (g0 + 1) * D1],
                            pp[:, 0:R],
                            start=c == 0, stop=c == NCH - 1,
                        )
                        nc.tensor.matmul(
                            po1[:],
                            h_sb[:, c * F1 + g1 * D1:c * F1 + (g1 + 1) * D1],
                            pp[:, R:2 * R],
                            start=c == 0, stop=c == NCH - 1,
                        )
                        nc.tensor.matmul(
                            pz[:, 0:R], onesb_sb[:], pp[:, 0:R],
                            start=c == 0, stop=c == NCH - 1,
                        )
                        nc.tensor.matmul(
                            pz[:, R:2 * R], onesb_sb[:], pp[:, R:2 * R],
                            start=c == 0, stop=c == NCH - 1,
                        )
                    # normalize + ELU -> h1^T (bf16)
                    zr = npool.tile([128, 2 * R], FP32, tag="zr")
                    nc.vector.reciprocal_approx_fast(zr[:], pz[:])
                    for k, g, po in ((0, g0, po0), (1, g1, po1)):
                        pre = npool.tile([128, R], FP32, tag="pre")
                        nc.vector.tensor_mul(
                            pre[:], po[:], zr[:, k * R:(k + 1) * R]
                        )
                        r = npool.tile([128, R], FP32, tag="r")
                        nc.scalar.activation(r[:], pre[:], AF.Relu, scale=-1.0)
                        t = npool.tile([128, R], FP32, tag="t")
                        nc.scalar.activation(t[:], r[:], AF.Exp, scale=-1.0)
                        nc.vector.scalar_tensor_tensor(
                            h1T[:, g * R:(g + 1) * R], t[:], -1.0, pre[:],
                            op0=OP.add, op1=OP.max,
                        )

            # =============================================================
            # Phase D: layer-2 transform + AllGather of [h2 | fs2 | fd2]
            # =============================================================
            with (
                tc.tile_pool(name="p2", bufs=2, space="PSUM") as p2p,
                tc.tile_pool(name="p2t", bufs=2, space="PSUM") as p2tp,
            ):
                for jt2 in range(R // 128):
                    ph2 = p2p.tile([128, 18], FP32, tag="ph2")
                    for g in range(H1):
                        nc.tensor.matmul(
                            ph2[:],
                            h1T[:, g * R + jt2 * 128:g * R + (jt2 + 1) * 128],
                            W2a_sb[:, g * 18:(g + 1) * 18],
                            start=g == 0, stop=g == H1 - 1,
                        )
                    nc.vector.tensor_copy(
                        h2P_sb[:, jt2 * 18:(jt2 + 1) * 18], ph2[:]
                    )
                    ps2 = p2tp.tile([1, 128], BF16, tag="ps2")
                    nc.tensor.transpose(
                        ps2[:], h2P_sb[:, jt2 * 18 + 16:jt2 * 18 + 17], idb_sb[:]
                    )
                    nc.vector.tensor_copy(
                        fs2row[0:1, jt2 * 128:(jt2 + 1) * 128], ps2[:]
                    )

                # local-only prep before the collective, hides in gather wait
                a2row = npool_tile = None
                with tc.tile_pool(name="h2s", bufs=1) as h2p:
                    a2row = h2p.tile([1, R], BF16, tag="a2row")
                    nc.scalar.activation(a2row[:], fs2row[:], AF.Copy)
                    pab2 = p2tp.tile([128, R], FP32, tag="pab2")
                    nc.tensor.matmul(
                        pab2[:], onesb_sb[0:1, :], a2row[0:1, :],
                        start=True, stop=True,
                    )
                    nc.scalar.activation(fsb2[:], pab2[:], AF.Copy)
                    nc.scalar.activation(
                        abc2[:], pab2[:], AF.Exp, scale=1.0 - ALPHA
                    )

                    nc.sync.dma_start(h2loc[:], h2P_sb[:])
                    nc.gpsimd.collective_compute(
                        "AllGather",
                        OP.bypass,
                        replica_groups=[list(range(NCORES))],
                        ins=[h2loc[:].opt()],
                        outs=[h2allP_d[:].opt()],
                    )
                    # load gathered [8*128, 72] -> [128, 8*72] (= [128, 576])
                    nc.sync.dma_start(
                        h2all_sb[:].rearrange("p (k f) -> p k f", k=NCORES),
                        h2allP_d[:].rearrange("(k p) f -> p k f", p=128),
                    )
                    # fd2 / vd2 / efd2 from col 17 of each 18-block; col 16 -> 1
                    h2v = h2all_sb[:].rearrange("p (m o) -> p m o", o=18)
                    nc.vector.tensor_copy(fd2_sb[:], h2v[:, :, 17])
                    nc.scalar.activation(
                        vd2_sb[:], fd2_sb[:], AF.Exp, scale=ALPHA
                    )
                    nc.scalar.activation(efd2_sb[:], fd2_sb[:], AF.Exp)
                    nc.vector.memset(h2v[:, :, 16], 1.0)

            # =============================================================
            # Phase E: layer-2 attention + ELU + log_softmax
            # =============================================================
            with (
                tc.tile_pool(name="acc2", bufs=1, space="PSUM") as acc2,
                tc.tile_pool(name="sc2", bufs=4) as spool2,
                tc.tile_pool(name="fin", bufs=2) as fpool,
                tc.tile_pool(name="pfin", bufs=2, space="PSUM") as pfp2,
            ):
                po2 = acc2.tile([18, R], FP32, tag="o2")
                for c in range(NCH):
                    if _act_route2(c):
                        t12 = spool2.tile([128, R], BF16, tag="t12")
                        nc.scalar.activation(
                            t12[:], fsb2[:], AF.Exp,
                            bias=fd2_sb[:, c:c + 1], scale=1.0 - ALPHA,
                        )
                        src2 = t12
                    else:
                        u2 = spool2.tile([128, R], BF16, tag="t12")
                        nc.vector.tensor_scalar_mul(
                            u2[:], abc2[:], efd2_sb[:, c:c + 1]
                        )
                        src2 = u2
                    p2t = spool2.tile([128, R], BF16, tag="p2m")
                    nc.vector.scalar_tensor_tensor(
                        p2t[:], src2[:], vd2_sb[:, c:c + 1],
                        adjT_sb[:, c * R:(c + 1) * R],
                        op0=OP.max, op1=OP.mult,
                    )
                    nc.tensor.matmul(
                        po2[:], h2all_sb[:, c * 18:(c + 1) * 18], p2t[:],
                        start=c == 0, stop=c == NCH - 1,
                    )
                po2sb = fpool.tile([18, R], FP32, tag="po2sb")
                nc.scalar.activation(po2sb[:], po2[:], AF.Copy)
                zrow = fpool.tile([1, R], FP32, tag="zrow")
                nc.sync.dma_start(zrow[0:1, :], po2sb[16:17, :])
                zrowi = fpool.tile([1, R], FP32, tag="zrowi")
                nc.vector.reciprocal_approx_fast(zrowi[:], zrow[:])
                zrowb = fpool.tile([1, R], BF16, tag="zrowb")
                nc.scalar.activation(zrowb[:], zrowi[:], AF.Copy)
                pzb = pfp2.tile([16, R], FP32, tag="pzb")
                nc.tensor.matmul(
                    pzb[:], onesb_sb[0:1, 0:16], zrowb[0:1, :],
                    start=True, stop=True,
                )
                zr2 = fpool.tile([16, R], FP32, tag="zr2")
                nc.vector.tensor_copy(zr2[:], pzb[:])
                pre2 = fpool.tile([16, R], FP32, tag="pre2")
                nc.vector.tensor_mul(pre2[:], po2sb[0:16, :], zr2[:])
                r2 = fpool.tile([16, R], FP32, tag="r2")
                nc.scalar.activation(r2[:], pre2[:], AF.Relu, scale=-1.0)
                t2 = fpool.tile([16, R], FP32, tag="t2")
                nc.scalar.activation(t2[:], r2[:], AF.Exp, scale=-1.0)
                elu2 = fpool.tile([16, R], FP32, tag="elu2")
                nc.vector.scalar_tensor_tensor(
                    elu2[:], t2[:], -1.0, pre2[:], op0=OP.add, op1=OP.max
                )
                # transpose to natural [i, o2] then log_softmax over free dim
                for it in range(R // 128):
                    pn = pfp2.tile([128, 16], FP32, tag="pn")
                    nc.tensor.transpose(
                        pn[:], elu2[:, it * 128:(it + 1) * 128], idf_sb[:]
                    )
                    nmx = fpool.tile([128, 1], FP32, tag="nmx")
                    nc.vector.tensor_reduce(
                        nmx[:], pn[:], AX.X, OP.max, negate=True
                    )
                    ex = fpool.tile([128, 16], FP32, tag="ex")
                    s = fpool.tile([128, 1], FP32, tag="s")
                    nc.scalar.activation(
                        ex[:], pn[:], AF.Exp, bias=nmx[:, 0:1], accum_out=s[:, 0:1]
                    )
                    lg = fpool.tile([128, 1], FP32, tag="lg")
                    nc.scalar.activation(lg[:], s[:], AF.Ln)
                    nc.vector.tensor_scalar(
                        outbuf[:, it * 16:(it + 1) * 16], pn[:],
                        nmx[:, 0:1], lg[:, 0:1],
                        op0=OP.add, op1=OP.subtract,
                    )
                nc.sync.dma_start(out_d[:], outbuf[:])

    nc.compile()
    return nc


def _get_nc():
    if "nc" not in _BUILD_CACHE:
        _BUILD_CACHE["nc"] = _build_nc()
    return _BUILD_CACHE["nc"]


def _pack_fmajor(a, nfc=NFC):
    """[nfc*128, C] -> [128, nfc*C] partition-major packing."""
    c = a.shape[1]
    return np.ascontiguousarray(
        a.reshape(nfc, 128, c).transpose(1, 0, 2).reshape(128, nfc * c)
    )


def _prep_inputs(x, adj, W1, a_src1, a_dst1, W2, a_src2, a_dst2):
    bf16 = ml_dtypes.bfloat16
    f32 = np.float32
    x = np.asarray(x, f32)
    adj = np.asarray(adj, f32)
    W1 = np.asarray(W1, f32)
    W2 = np.asarray(W2, f32)
    a_src1 = np.asarray(a_src1, f32)
    a_dst1 = np.asarray(a_dst1, f32)
    a_src2 = np.asarray(a_src2, f32)
    a_dst2 = np.asarray(a_dst2, f32)

    W1f = np.ascontiguousarray(W1.reshape(F_IN, F1))
    # folded score vectors: f_src[h] = x @ (W1[:,h,:] @ a_src1[h])
    wsrc = np.stack([W1[:, h, :] @ a_src1[h] for h in range(H1)], axis=1)
    wdst = np.stack([W1[:, h, :] @ a_dst1[h] for h in range(H1)], axis=1)
    wfold = np.concatenate([wsrc, wdst], axis=1)          # [512, 16]
    W2f = np.ascontiguousarray(W2.reshape(F1, D2))
    W2a = np.zeros((F1, 18), f32)
    W2a[:, :D2] = W2f
    W2a[:, 16] = W2f @ a_src2[0]
    W2a[:, 17] = W2f @ a_dst2[0]

    xT = np.ascontiguousarray(x.T)                        # [512, 4096]
    # xP[p, q*4096 + fc*1024 + jj] = x[q*1024+jj, fc*128+p]
    xP = np.ascontiguousarray(
        x.reshape(4, 1024, NFC, 128).transpose(3, 0, 2, 1).reshape(128, NFC * N)
    )
    ident = np.eye(128, dtype=f32)

    shared = {
        "xP": xP.astype(bf16),
        "W1P": _pack_fmajor(W1f).astype(bf16),
        "wfold": _pack_fmajor(wfold).astype(bf16),
        "onesb": np.ones((128, 128), bf16),
        "idb": ident.astype(bf16),
        "idf16": np.eye(16, dtype=f32),
        "W2aP": _pack_fmajor(W2a, nfc=H1).astype(bf16),
    }
    in_maps = []
    for c in range(NCORES):
        blkslice = slice(c * R, (c + 1) * R)
        m = dict(shared)
        adjT = np.ascontiguousarray(adj[blkslice, :].T)   # [4096, 512]
        m["adjP"] = _pack_fmajor(adjT, nfc=NCH).astype(bf16)
        xo = np.ascontiguousarray(x[blkslice, :].T)       # [512, 512]
        m["xoP"] = _pack_fmajor(xo).astype(bf16)
        in_maps.append(m)
    return in_maps


def kernel(x, adj, W1, a_src1, a_dst1, W2, a_src2, a_dst2, _trace=False):
    from concourse.bass_utils import run_bass_kernel_spmd

    nc = _get_nc()
    in_maps = _prep_inputs(x, adj, W1, a_src1, a_dst1, W2, a_src2, a_dst2)
    res = run_bass_kernel_spmd(nc, in_maps, list(range(NCORES)), trace=_trace)
    blocks = []
    for c in range(NCORES):
        ob = np.asarray(res.results[c]["out"])            # [128, 4*16]
        blocks.append(
            ob.reshape(128, R // 128, D2).transpose(1, 0, 2).reshape(R, D2)
        )
    out = np.concatenate(blocks, axis=0)
    kernel.last_results = res
    return out.astype(np.float32)
